# revision 26
# baseline (speedup 1.0000x reference)
"""Trainium2 Bass kernel for nn_CrossAttentionFusion.

Math. With a single-token key/value axis, softmax over that axis is exactly
1.0, so each cross-attention path collapses to its V/out projections:

    z_i = (x_kv @ wv_i^T + bv_i) @ w_o_i^T + b_o_i = x_kv @ W_i^T + c_i
      with W_i = w_o_i @ wv_i,  c_i = w_o_i @ bv_i + b_o_i.

The LayerNorm folds through the output projection, and the projection folds
through the attention collapse: with z = [z1 | z2], G = [Wg1@W1 | Wg2@W2],
d2 = Wg @ c, wbar = row sums of Wg (Wg = w_proj * ln_g), b2 = w_proj@ln_b +
b_proj, per-row mean mu and rstd rs of (z + c):

    out = gelu(rs * (G @ x + d2 - mu * wbar) + b2)

so the full-precision y-path never needs z. z is only needed for the LN
statistics (mu, var), which tolerate fp8.

Device pipeline (per core, batch shard of 2048 rows, all transposed so the
feature dim lies on partitions):
  y-path:  yp = Gh@xh + Gh@xl + Gl@xh  (error-compensated fp8e4 DoubleRow
           matmuls, f32 PSUM; Gh = fp8(64G), Gl = fp8(64G - Gh), xh = fp8(x),
           xl = fp8(x - xh)) then a rank-1 f32r matmul adds mu*(-64wbar) +
           64*d2 in PSUM.
  z-path:  zp = fp8(32W) @ xh (DoubleRow), squared into fp8 by ScalarE
           (Square with scale/bias handles the z bias), S1 = r^T xh and
           S2 = 1^T zq as DoubleRow matvecs.
  stats:   mu, rs/64 on DVE/ScalarE vectors; rs broadcast across partitions
           via an f32r outer-product matmul.
  epilogue: DVE multiplies by rs/64, ScalarE applies b2 + exact-erf GELU.
Host does the transposes, weight folds and fp8 splits.
"""

import os
import sys

sys.path.insert(0, "/opt/trn_rl_repo")

import ml_dtypes
import numpy as np

import concourse.bass as bass
import concourse.mybir as mybir
import concourse.tile as tile
from concourse.bass import ts
from concourse.bass_utils import run_bass_kernel_spmd

N_CORES = 8
B = 16384
D = 1024
BC = B // N_CORES          # batch rows per core
NCHUNK = 512               # batch columns processed per chunk
NCH = BC // NCHUNK         # chunks per core
KT1 = D // 128             # k-tiles per input (8)
KT3 = 2 * D // 128         # k-tiles of the concatenated input (16)
MT1 = 2 * D // 128         # m-tiles of z (16)
MT3 = D // 128             # m-tiles of y (8)
LN_EPS = 1e-5

SX = 64.0                  # fp8 scale on G
SW = 32.0                  # fp8 scale on W
SQ = 1.0 / 16.0            # ScalarE scale before squaring: (z*SW)*SQ = 2z

F8 = mybir.dt.float8e4
F32 = mybir.dt.float32
F32R = mybir.dt.float32r
BF16 = mybir.dt.bfloat16
nf8 = ml_dtypes.float8_e4m3   # TRN-compatible e4m3 (max +-240)

ALU = mybir.AluOpType
AF = mybir.ActivationFunctionType
DR = mybir.MatmulPerfMode.DoubleRow


def split_multi_waits(nc):
    """This walrus build only honors one sync-wait per instruction. Move any
    extra waits onto same-engine NOPs inserted immediately before."""
    for f in nc.m.functions:
        for bb in f.blocks:
            new_insts = []
            changed = False
            for inst in bb.instructions:
                si = inst.sync_info
                waits = list(si.on_wait) if si and si.on_wait else []
                if len(waits) > 1:
                    changed = True
                    for w in waits[:-1]:
                        nop = mybir.InstNoOp(
                            name=nc.get_next_instruction_name(), ins=[], outs=[]
                        )
                        nop.engine = inst.engine
                        nop.sync_info = mybir.SyncInfo(on_wait=[w], on_update=[])
                        nc.register_instruction(nop)
                        new_insts.append(nop)
                    si.on_wait = waits[-1:]
                new_insts.append(inst)
            if changed:
                bb.instructions[:] = new_insts


def build_program(repeat=1):
    nc = bass.Bass("TRN2", target_bir_lowering=False, debug=False)

    xhu = nc.dram_tensor("xhu", [D, BC], F8, kind="ExternalInput").ap()
    xhm = nc.dram_tensor("xhm", [D, BC], F8, kind="ExternalInput").ap()
    xlu = nc.dram_tensor("xlu", [D, BC], F8, kind="ExternalInput").ap()
    xlm = nc.dram_tensor("xlm", [D, BC], F8, kind="ExternalInput").ap()
    gh = nc.dram_tensor("gh", [2 * D, D], F8, kind="ExternalInput").ap()
    gl = nc.dram_tensor("gl", [2 * D, D], F8, kind="ExternalInput").ap()
    w1h = nc.dram_tensor("w1h", [D, D], F8, kind="ExternalInput").ap()
    w2h = nc.dram_tensor("w2h", [D, D], F8, kind="ExternalInput").ap()
    rr = nc.dram_tensor("rr", [2 * D, 16], F8, kind="ExternalInput").ap()
    lt = nc.dram_tensor("lt", [1, 2 * MT3, 128], F8, kind="ExternalInput").ap()
    cb2 = nc.dram_tensor("cb2", [128, MT3], F32, kind="ExternalInput").ap()
    csq = nc.dram_tensor("csq", [128, MT1], F32, kind="ExternalInput").ap()
    scv = nc.dram_tensor("scv", [1, 1], F32, kind="ExternalInput").ap()
    out = nc.dram_tensor("outT", [D, BC], BF16, kind="ExternalOutput").ap()

    xhu3 = xhu.rearrange("(k p) n -> p k n", p=128)
    xhm3 = xhm.rearrange("(k p) n -> p k n", p=128)
    xlu3 = xlu.rearrange("(k p) n -> p k n", p=128)
    xlm3 = xlm.rearrange("(k p) n -> p k n", p=128)
    gh3 = gh.rearrange("(k p) m -> p k m", p=128)
    gl3 = gl.rearrange("(k p) m -> p k m", p=128)
    w1h3 = w1h.rearrange("(k p) m -> p k m", p=128)
    w2h3 = w2h.rearrange("(k p) m -> p k m", p=128)
    rr3 = rr.rearrange("(k p) m -> p k m", p=128)

    with tile.TileContext(nc) as tc:
        with (
            tc.tile_pool(name="wconst", bufs=1) as wconst,
            tc.tile_pool(name="xin", bufs=2) as xin,
            tc.tile_pool(name="zq", bufs=2) as zqp,
            tc.tile_pool(name="ysb", bufs=2) as ysbp,
            tc.tile_pool(name="scal", bufs=2) as scal,
            tc.tile_pool(name="outp", bufs=4) as outp,
            tc.tile_pool(name="opool", bufs=8) as opool,
            tc.tile_pool(name="zps", bufs=3, space="PSUM") as zps,
            tc.tile_pool(name="yps", bufs=2, space="PSUM") as yps,
            tc.tile_pool(name="s1ps", bufs=1, space="PSUM") as s1ps,
            tc.tile_pool(name="s2ps", bufs=1, space="PSUM") as s2ps,
            tc.tile_pool(name="bps", bufs=1, space="PSUM") as bps,
        ):
            # --- resident constants ---
            w1_sb = wconst.tile([128, KT1, D], F8)
            w2_sb = wconst.tile([128, KT1, D], F8)
            gh_sb = wconst.tile([128, KT3, D], F8)
            gl_sb = wconst.tile([128, KT3, D], F8)
            rr_sb = wconst.tile([128, KT3, 16], F8)
            lt_sb = wconst.tile([1, 2 * MT3, 128], F8)
            cb2_sb = wconst.tile([128, MT3], F32)
            csq_sb = wconst.tile([128, MT1], F32)
            sc_sb = wconst.tile([1, 1], F32)
            ones2 = wconst.tile([128, 2, 16], F8)
            nc.vector.memset(ones2[:], 1.0)
            onesf = wconst.tile([1, 128], BF16)
            nc.vector.memset(onesf[:], 1.0)
            eps_sb = wconst.tile([1, 1], F32)
            nc.vector.memset(eps_sb[:], SX * SX * LN_EPS)

            for rep in range(repeat):
              for ci in range(NCH):
                n0 = ci * NCHUNK
                nsl = slice(n0, n0 + NCHUNK)

                xhu_sb = xin.tile([128, KT1, NCHUNK], F8, tag="xhu")
                nc.sync.dma_start(xhu_sb[:], xhu3[:, :, nsl])
                if rep == 0 and ci == 0:
                    nc.sync.dma_start(rr_sb[:], rr3[:])
                    for kt in range(KT1 - 1, 3, -1):
                        ke = min(D, (kt // 2 + 1) * 256)
                        nc.sync.dma_start(
                            w1_sb[:, kt, 0:ke], w1h3[:, kt, 0:ke]
                        )
                xhm_sb = xin.tile([128, KT1, NCHUNK], F8, tag="xhm")
                nc.sync.dma_start(xhm_sb[:], xhm3[:, :, nsl])
                if rep == 0 and ci == 0:
                    for kt in range(KT1 - 1, 3, -1):
                        ke = min(D, (kt // 2 + 1) * 256)
                        nc.sync.dma_start(
                            w2_sb[:, kt, 0:ke], w2h3[:, kt, 0:ke]
                        )
                    for kt in range(3, -1, -1):
                        ke = min(D, (kt // 2 + 1) * 256)
                        nc.sync.dma_start(
                            w1_sb[:, kt, 0:ke], w1h3[:, kt, 0:ke]
                        )
                        nc.sync.dma_start(
                            w2_sb[:, kt, 0:ke], w2h3[:, kt, 0:ke]
                        )
                    nc.sync.dma_start(lt_sb[:], lt[:])
                    nc.sync.dma_start(cb2_sb[:], cb2[:])
                    nc.sync.dma_start(csq_sb[:], csq[:])
                    nc.sync.dma_start(sc_sb[:], scv[:])
                    nc.sync.dma_start(gh_sb[:, :, 0:512], gh3[:, :, 0:512])
                    nc.sync.dma_start(gl_sb[:, :, 0:512], gl3[:, :, 0:512])
                xlu_sb = xin.tile([128, KT1, NCHUNK], F8, tag="xlu")
                nc.sync.dma_start(xlu_sb[:], xlu3[:, :, nsl])
                xlm_sb = xin.tile([128, KT1, NCHUNK], F8, tag="xlm")
                nc.sync.dma_start(xlm_sb[:], xlm3[:, :, nsl])
                if rep == 0 and ci == 0:
                    nc.sync.dma_start(gh_sb[:, :, 512:D], gh3[:, :, 512:D])
                    nc.sync.dma_start(gl_sb[:, :, 512:D], gl3[:, :, 512:D])

                # --- z-path: zp = (SW*W) @ x in DoubleRow fp8; square on
                # ScalarE straight out of PSUM into packed fp8 tiles ---
                zq_sb = zqp.tile([128, MT1, NCHUNK], F8)

                def z_tile(mg):
                    half, m = divmod(mg, D // 128)
                    xh_sb, w_sb = ((xhu_sb, w1_sb), (xhm_sb, w2_sb))[half]
                    zp = zps.tile([128, NCHUNK], F32, tag="zp")
                    p0 = m // 2   # k-tile pairs below the diagonal are zero
                    for t in range(p0, KT1 // 2):
                        nc.tensor.matmul(
                            zp[:],
                            lhsT=w_sb[:, 2 * t : 2 * t + 2, ts(m, 128)],
                            rhs=xh_sb[:, 2 * t : 2 * t + 2, :],
                            start=(t == p0),
                            stop=(t == KT1 // 2 - 1),
                            perf_mode=DR,
                        )
                    # zq = ((zp + SW*c) * SQ)^2, quantized to fp8
                    nc.scalar.activation(
                        zq_sb[:, mg, :],
                        zp[:],
                        AF.Square,
                        bias=csq_sb[:, mg : mg + 1],
                        scale=SQ,
                    )

                # --- S1 = r^T x (DoubleRow matvecs over x) ---
                s1p = s1ps.tile([1, NCHUNK], F32, tag="s1")

                def s1_part(t):
                    xh_sb = xhu_sb if t < KT1 // 2 else xhm_sb
                    tt = t if t < KT1 // 2 else t - KT1 // 2
                    nc.tensor.matmul(
                        s1p[:],
                        lhsT=rr_sb[:, 2 * t : 2 * t + 2, 0:1],
                        rhs=xh_sb[:, 2 * tt : 2 * tt + 2, :],
                        start=(t == 0),
                        stop=(t == KT3 // 2 - 1),
                        perf_mode=DR,
                    )

                # 64*mu into slot 0 of the DoubleRow rank-1 rhs; slot 1 = 1
                mu2 = scal.tile([1, 2, NCHUNK], F8, tag="mu2")

                def mu_ops():
                    nc.vector.memset(mu2[:, 1, :], 1.0)
                    nc.vector.tensor_scalar(
                        out=mu2[:, 0, :],
                        in0=s1p[:],
                        scalar1=SX / (2 * D * SW),
                        scalar2=sc_sb[:],
                        op0=ALU.mult,
                        op1=ALU.add,
                    )

                # --- y-path + staging copies to SBUF ---
                ysb = ysbp.tile([128, MT3, NCHUNK], F32)
                yp_last = {}

                def y_mtile(m):
                    yp = yps.tile([128, NCHUNK], F32, tag="yp")
                    first = True
                    for ga_sb, xa in (
                        (gh_sb, (xhu_sb, xhm_sb)),
                        (gl_sb, (xhu_sb, xhm_sb)),
                        (gh_sb, (xlu_sb, xlm_sb)),
                    ):
                        for t in range(KT3 // 2):
                            x_sb = xa[0] if t < KT1 // 2 else xa[1]
                            tt = t if t < KT1 // 2 else t - KT1 // 2
                            nc.tensor.matmul(
                                yp[:],
                                lhsT=ga_sb[:, 2 * t : 2 * t + 2, ts(m, 128)],
                                rhs=x_sb[:, 2 * tt : 2 * tt + 2, :],
                                start=first,
                                stop=False,
                                perf_mode=DR,
                            )
                            first = False
                    # rank-1 update: yp += (SX*mu)*(-wbar_m) + 1*(SX*d2_m)
                    nc.tensor.matmul(
                        yp[:],
                        lhsT=lt_sb[:, 2 * m : 2 * m + 2, :],
                        rhs=mu2[:],
                        start=False,
                        stop=True,
                        perf_mode=DR,
                        skip_group_check=True,
                    )
                    if m < 6:
                        nc.vector.tensor_copy(out=ysb[:, m, :], in_=yp[:])
                    else:
                        yp_last[m] = yp

                if rep == 0 and ci == 0:
                    # weights still streaming in: follow the DMA arrival order
                    for mg in (7, 6, 5, 4):
                        z_tile(mg)
                    for t in range(4):
                        s1_part(t)
                    for mg in (15, 14, 13, 12):
                        z_tile(mg)
                    for t in range(4, KT3 // 2):
                        s1_part(t)
                    for mg in (3, 2, 1, 0, 11, 10, 9, 8):
                        z_tile(mg)
                    mu_ops()
                    for m in range(4):
                        y_mtile(m)
                else:
                    # interleave z tiles between y tiles so ScalarE squares
                    # never gate the zps PSUM banks
                    for mg in range(4):
                        z_tile(mg)
                    for t in range(4):
                        s1_part(t)
                    for mg in range(4, 8):
                        z_tile(mg)
                    for t in range(4, 8):
                        s1_part(t)
                    mu_ops()
                    y_mtile(0)
                    for mg in range(8, 10):
                        z_tile(mg)
                    y_mtile(1)
                    for mg in range(10, 12):
                        z_tile(mg)
                    y_mtile(2)
                    for mg in range(12, 14):
                        z_tile(mg)
                    y_mtile(3)
                    for mg in range(14, 16):
                        z_tile(mg)

                # --- S2 = 1^T zq (DoubleRow matvecs over squared z) ---
                s2p = s2ps.tile([1, NCHUNK], F32, tag="s2")
                for t in range(MT1 // 2):
                    nc.tensor.matmul(
                        s2p[:],
                        lhsT=ones2[:, :, 0:1],
                        rhs=zq_sb[:, 2 * t : 2 * t + 2, :],
                        start=(t == 0),
                        stop=(t == MT1 // 2 - 1),
                        perf_mode=DR,
                    )

                # --- mu^2, var, rs/SX while the rest of the y-path runs ---
                musq = scal.tile([1, NCHUNK], F32, tag="musq")
                nc.scalar.activation(
                    musq[:], mu2[:, 0, :], AF.Square, scale=1.0 / SX
                )
                var = scal.tile([1, NCHUNK], F32, tag="var")
                nc.vector.scalar_tensor_tensor(
                    out=var[:],
                    in0=s2p[:],
                    scalar=1.0 / (2 * D * (SW * SQ) ** 2),
                    in1=musq[:],
                    op0=ALU.mult,
                    op1=ALU.subtract,
                )
                sd = scal.tile([1, NCHUNK], F32, tag="sd")
                nc.scalar.activation(
                    sd[:], var[:], AF.Sqrt, bias=eps_sb[:], scale=SX * SX
                )
                rstf = scal.tile([1, NCHUNK], F32, tag="rstf")
                nc.vector.reciprocal(rstf[:], sd[:])
                rst = scal.tile([1, NCHUNK], BF16, tag="rst")
                nc.vector.tensor_copy(out=rst[:], in_=rstf[:])

                y_mtile(4)
                y_mtile(5)

                # --- broadcast rs/SX across partitions (outer product) ---
                bp = bps.tile([128, NCHUNK], F32, tag="bp")
                nc.tensor.matmul(
                    bp[:],
                    lhsT=onesf[:],
                    rhs=rst[:],
                    start=True,
                    stop=True,
                )

                bp_sb = outp.tile([128, NCHUNK], F32, tag="bp_sb")
                nc.vector.tensor_copy(out=bp_sb[:], in_=bp[:])

                # --- epilogue: scale by rs/SX, bias + exact-erf GELU; the
                # last two y m-tiles overlap the first six epilogues ---
                def epi(m):
                    t1 = outp.tile([128, NCHUNK], F32, tag="t1")
                    src_ap = ysb[:, m, :] if m < 6 else yp_last[m][:]
                    nc.vector.tensor_mul(t1[:], src_ap, bp_sb[:])
                    o_sb = opool.tile([128, NCHUNK], BF16, tag="o")
                    nc.scalar.activation(
                        o_sb[:], t1[:], AF.Gelu, bias=cb2_sb[:, m : m + 1]
                    )
                    nc.sync.dma_start(out[ts(m, 128), nsl], o_sb[:])

                epi(0)
                epi(1)
                epi(2)
                y_mtile(6)
                epi(3)
                epi(4)
                epi(5)
                epi(6)
                y_mtile(7)
                if ci == NCH - 1 and rep == repeat - 1:
                    for hs in (slice(0, 256), slice(256, 512)):
                        t1 = outp.tile([128, NCHUNK], F32, tag="t1")
                        nc.vector.tensor_mul(
                            t1[:, hs], yp_last[7][:, hs], bp_sb[:, hs]
                        )
                        o_sb = opool.tile([128, NCHUNK], BF16, tag="o")
                        nc.scalar.activation(
                            o_sb[:, hs], t1[:, hs], AF.Gelu,
                            bias=cb2_sb[:, 7:8],
                        )
                        nc.sync.dma_start(
                            out[ts(7, 128), n0 + hs.start : n0 + hs.stop],
                            o_sb[:, hs],
                        )
                else:
                    epi(7)
    split_multi_waits(nc)
    return nc


def q8(a):
    return np.clip(np.asarray(a, np.float32), -240.0, 240.0).astype(nf8)


def fold_weights(inputs):
    f32 = np.float32
    d = D
    w_qkv1 = np.asarray(inputs["w_qkv1"], f32)
    w_qkv2 = np.asarray(inputs["w_qkv2"], f32)
    b_qkv1 = np.asarray(inputs["b_qkv1"], f32)
    b_qkv2 = np.asarray(inputs["b_qkv2"], f32)
    w_o1 = np.asarray(inputs["w_o1"], f32)
    w_o2 = np.asarray(inputs["w_o2"], f32)
    b_o1 = np.asarray(inputs["b_o1"], f32)
    b_o2 = np.asarray(inputs["b_o2"], f32)
    w_proj = np.asarray(inputs["w_proj"], f32)
    b_proj = np.asarray(inputs["b_proj"], f32)
    g = np.asarray(inputs["ln_g"], f32)
    lb = np.asarray(inputs["ln_b"], f32)

    wv1, bv1 = w_qkv1[2 * d :], b_qkv1[2 * d :]
    wv2, bv2 = w_qkv2[2 * d :], b_qkv2[2 * d :]
    W1 = w_o1 @ wv1
    c1 = w_o1 @ bv1 + b_o1
    W2 = w_o2 @ wv2
    c2 = w_o2 @ bv2 + b_o2
    Wg = w_proj * g[None, :]
    G = np.concatenate([Wg[:, :d] @ W1, Wg[:, d:] @ W2], axis=1)  # [d, 2d]
    cvec = np.concatenate([c1, c2])
    d2 = Wg @ cvec
    wbar = Wg.sum(axis=1)
    b2 = w_proj @ lb + b_proj

    Gs = G.T * SX                      # [2d, d] lhsT layout
    gh = q8(Gs)
    gl = q8(Gs - gh.astype(f32))

    def chol_factor(W):
        # sum_f z_f^2 = x^T (W^T W) x = ||L^T x||^2; lower-tri L halves the
        # k-tiles of the stats matmul. Jitter guards near-singular C.
        C = W.T.astype(np.float64) @ W.astype(np.float64)
        scale = np.trace(C) / d
        for jit in (0.0, 1e-12, 1e-9, 1e-6):
            try:
                return np.linalg.cholesky(C + jit * scale * np.eye(d))
            except np.linalg.LinAlgError:
                continue
        raise np.linalg.LinAlgError("cholesky failed")

    w1h = q8(chol_factor(W1).astype(f32) * SW)
    w2h = q8(chol_factor(W2).astype(f32) * SW)
    rrv = np.zeros((2 * d, 16), f32)
    rrv[:d, 0] = W1.sum(axis=0) * SW
    rrv[d:, 0] = W2.sum(axis=0) * SW
    ltv = np.empty((1, 2 * MT3, 128), f32)
    ltv[0, 0::2, :] = (-wbar).reshape(MT3, 128)
    ltv[0, 1::2, :] = (SX * d2).reshape(MT3, 128)
    return {
        "gh": gh,
        "gl": gl,
        "w1h": w1h,
        "w2h": w2h,
        "rr": q8(rrv),
        "lt": q8(ltv),
        "cb2": np.ascontiguousarray(b2.reshape(MT3, 128).T).astype(f32),
        "csq": np.ascontiguousarray(
            (2.0 * cvec).reshape(MT1, 128).T
        ).astype(f32),
        "scv": np.full((1, 1), SX * cvec.sum() / (2 * d), f32),
        "_czero": not np.any(cvec),
    }


_CACHED_NC = None


def _get_program(czero=True):
    global _CACHED_NC
    if _CACHED_NC is None:
        _CACHED_NC = build_program()
    return _CACHED_NC


def run(inputs, trace=False):
    """Build per-core shards, run on 8 cores, return (full_out, results)."""
    x_u = np.asarray(inputs["x_u"], np.float32)
    x_m = np.asarray(inputs["x_m"], np.float32)
    shared = fold_weights(inputs)
    czero = shared.pop("_czero")
    xuT = np.ascontiguousarray(x_u.T)  # [D, B] f32
    xmT = np.ascontiguousarray(x_m.T)
    xhuT = q8(xuT)
    xhmT = q8(xmT)
    xluT = q8(xuT - xhuT.astype(np.float32))
    xlmT = q8(xmT - xhmT.astype(np.float32))

    in_maps = []
    for c in range(N_CORES):
        sl = slice(c * BC, (c + 1) * BC)
        m = dict(shared)
        m["xhu"] = np.ascontiguousarray(xhuT[:, sl])
        m["xhm"] = np.ascontiguousarray(xhmT[:, sl])
        m["xlu"] = np.ascontiguousarray(xluT[:, sl])
        m["xlm"] = np.ascontiguousarray(xlmT[:, sl])
        in_maps.append(m)

    nc = _get_program(czero=czero)
    res = run_bass_kernel_spmd(
        nc, in_maps, list(range(N_CORES)), trace=trace
    )
    out = np.empty((B, D), np.float32)
    for c in range(N_CORES):
        out[c * BC : (c + 1) * BC, :] = res.results[c]["outT"].T.astype(np.float32)
    return out, res


def kernel(**inputs) -> np.ndarray:
    out, _ = run(inputs, trace=False)
    return out


# revision 27
# speedup vs baseline: 1.0651x; 1.0651x over previous
"""Trainium2 Bass kernel for nn_CrossAttentionFusion.

Math. With a single-token key/value axis, softmax over that axis is exactly
1.0, so each cross-attention path collapses to its V/out projections:

    z_i = (x_kv @ wv_i^T + bv_i) @ w_o_i^T + b_o_i = x_kv @ W_i^T + c_i
      with W_i = w_o_i @ wv_i,  c_i = w_o_i @ bv_i + b_o_i.

The LayerNorm folds through the output projection, and the projection folds
through the attention collapse: with z = [z1 | z2], G = [Wg1@W1 | Wg2@W2],
d2 = Wg @ c, wbar = row sums of Wg (Wg = w_proj * ln_g), b2 = w_proj@ln_b +
b_proj, per-row mean mu and rstd rs of (z + c):

    out = gelu(rs * (G @ x + d2 - mu * wbar) + b2)

so the full-precision y-path never needs z. z is only needed for the LN
statistics (mu, var), which tolerate fp8.

Device pipeline (per core, batch shard of 2048 rows, all transposed so the
feature dim lies on partitions):
  y-path:  yp = Gh@xh + Gh@xl + Gl@xh  (error-compensated fp8e4 DoubleRow
           matmuls, f32 PSUM; Gh = fp8(64G), Gl = fp8(64G - Gh), xh = fp8(x),
           xl = fp8(x - xh)) then a rank-1 f32r matmul adds mu*(-64wbar) +
           64*d2 in PSUM.
  z-path:  zp = fp8(32W) @ xh (DoubleRow), squared into fp8 by ScalarE
           (Square with scale/bias handles the z bias), S1 = r^T xh and
           S2 = 1^T zq as DoubleRow matvecs.
  stats:   mu, rs/64 on DVE/ScalarE vectors; rs broadcast across partitions
           via an f32r outer-product matmul.
  epilogue: DVE multiplies by rs/64, ScalarE applies b2 + exact-erf GELU.
Host does the transposes, weight folds and fp8 splits.
"""

import os
import sys

sys.path.insert(0, "/opt/trn_rl_repo")

import ml_dtypes
import numpy as np

import concourse.bass as bass
import concourse.mybir as mybir
import concourse.tile as tile
from concourse.bass import ts
from concourse.bass_utils import run_bass_kernel_spmd

N_CORES = 8
B = 16384
D = 1024
BC = B // N_CORES          # batch rows per core
NCHUNK = 512               # batch columns processed per chunk
NCH = BC // NCHUNK         # chunks per core
KT1 = D // 128             # k-tiles per input (8)
KT3 = 2 * D // 128         # k-tiles of the concatenated input (16)
MT1 = 2 * D // 128         # m-tiles of z (16)
MT3 = D // 128             # m-tiles of y (8)
LN_EPS = 1e-5

SX = 64.0                  # fp8 scale on G
SW = 32.0                  # fp8 scale on W
SQ = 1.0 / 16.0            # ScalarE scale before squaring: (z*SW)*SQ = 2z

F8 = mybir.dt.float8e4
F32 = mybir.dt.float32
F32R = mybir.dt.float32r
BF16 = mybir.dt.bfloat16
nf8 = ml_dtypes.float8_e4m3   # TRN-compatible e4m3 (max +-240)

ALU = mybir.AluOpType
AF = mybir.ActivationFunctionType
DR = mybir.MatmulPerfMode.DoubleRow


def split_multi_waits(nc):
    """This walrus build only honors one sync-wait per instruction. Move any
    extra waits onto same-engine NOPs inserted immediately before."""
    for f in nc.m.functions:
        for bb in f.blocks:
            new_insts = []
            changed = False
            for inst in bb.instructions:
                si = inst.sync_info
                waits = list(si.on_wait) if si and si.on_wait else []
                if len(waits) > 1:
                    changed = True
                    for w in waits[:-1]:
                        nop = mybir.InstNoOp(
                            name=nc.get_next_instruction_name(), ins=[], outs=[]
                        )
                        nop.engine = inst.engine
                        nop.sync_info = mybir.SyncInfo(on_wait=[w], on_update=[])
                        nc.register_instruction(nop)
                        new_insts.append(nop)
                    si.on_wait = waits[-1:]
                new_insts.append(inst)
            if changed:
                bb.instructions[:] = new_insts


def build_program(repeat=1):
    nc = bass.Bass("TRN2", target_bir_lowering=False, debug=False)

    xhu = nc.dram_tensor("xhu", [D, BC], F8, kind="ExternalInput").ap()
    xhm = nc.dram_tensor("xhm", [D, BC], F8, kind="ExternalInput").ap()
    xlu = nc.dram_tensor("xlu", [D, BC], F8, kind="ExternalInput").ap()
    xlm = nc.dram_tensor("xlm", [D, BC], F8, kind="ExternalInput").ap()
    gh = nc.dram_tensor("gh", [2 * D, D], F8, kind="ExternalInput").ap()
    gl = nc.dram_tensor("gl", [2 * D, D], F8, kind="ExternalInput").ap()
    w1h = nc.dram_tensor("w1h", [D, D], F8, kind="ExternalInput").ap()
    w2h = nc.dram_tensor("w2h", [D, D], F8, kind="ExternalInput").ap()
    rr = nc.dram_tensor("rr", [128, 16 * 16], F8, kind="ExternalInput").ap()
    lt = nc.dram_tensor("lt", [1, 2 * MT3 * 128], F8, kind="ExternalInput").ap()
    cpk = nc.dram_tensor("cpk", [128, MT3 + MT1 + 1], F32, kind="ExternalInput").ap()
    out = nc.dram_tensor("outT", [D, BC], BF16, kind="ExternalOutput").ap()

    xhu3 = xhu.rearrange("(k p) n -> p k n", p=128)
    xhm3 = xhm.rearrange("(k p) n -> p k n", p=128)
    xlu3 = xlu.rearrange("(k p) n -> p k n", p=128)
    xlm3 = xlm.rearrange("(k p) n -> p k n", p=128)
    gh3 = gh.rearrange("(k p) m -> p k m", p=128)
    gl3 = gl.rearrange("(k p) m -> p k m", p=128)
    w1h3 = w1h.rearrange("(k p) m -> p k m", p=128)
    w2h3 = w2h.rearrange("(k p) m -> p k m", p=128)


    with tile.TileContext(nc) as tc:
        with (
            tc.tile_pool(name="wconst", bufs=1) as wconst,
            tc.tile_pool(name="xin", bufs=2) as xin,
            tc.tile_pool(name="zq", bufs=2) as zqp,
            tc.tile_pool(name="ysb", bufs=2) as ysbp,
            tc.tile_pool(name="scal", bufs=2) as scal,
            tc.tile_pool(name="outp", bufs=4) as outp,
            tc.tile_pool(name="opool", bufs=8) as opool,
            tc.tile_pool(name="zps", bufs=3, space="PSUM") as zps,
            tc.tile_pool(name="yps", bufs=2, space="PSUM") as yps,
            tc.tile_pool(name="s1ps", bufs=1, space="PSUM") as s1ps,
            tc.tile_pool(name="s2ps", bufs=1, space="PSUM") as s2ps,
            tc.tile_pool(name="bps", bufs=1, space="PSUM") as bps,
        ):
            # --- resident constants ---
            w1_sb = wconst.tile([128, KT1, D], F8)
            w2_sb = wconst.tile([128, KT1, D], F8)
            gh_sb = wconst.tile([128, KT3, D], F8)
            gl_sb = wconst.tile([128, KT3, D], F8)
            rr_sb = wconst.tile([128, KT3 * 16], F8)
            rr_v = rr_sb[:].rearrange("p (k j) -> p k j", j=16)
            lt_sb = wconst.tile([1, 2 * MT3 * 128], F8)
            lt_v = lt_sb[:].rearrange("a (k j) -> a k j", j=128)
            cpk_sb = wconst.tile([128, MT3 + MT1 + 1], F32)
            cb2_sb = cpk_sb[:, 0:MT3]
            csq_sb = cpk_sb[:, MT3 : MT3 + MT1]
            sc_sb = cpk_sb[0:1, MT3 + MT1 : MT3 + MT1 + 1]
            ones2 = wconst.tile([128, 2, 16], F8)
            nc.vector.memset(ones2[:], 1.0)
            onesf = wconst.tile([1, 128], BF16)
            nc.vector.memset(onesf[:], 1.0)
            eps_sb = wconst.tile([1, 1], F32)
            nc.vector.memset(eps_sb[:], SX * SX * LN_EPS)

            for rep in range(repeat):
              for ci in range(NCH):
                n0 = ci * NCHUNK
                nsl = slice(n0, n0 + NCHUNK)

                xhu_sb = xin.tile([128, KT1, NCHUNK], F8, tag="xhu")
                nc.sync.dma_start(xhu_sb[:], xhu3[:, :, nsl])
                if rep == 0 and ci == 0:
                    nc.sync.dma_start(rr_sb[:], rr[:])
                    nc.sync.dma_start(w1_sb[:, 4:8, :], w1h3[:, 4:8, :])
                xhm_sb = xin.tile([128, KT1, NCHUNK], F8, tag="xhm")
                nc.sync.dma_start(xhm_sb[:], xhm3[:, :, nsl])
                if rep == 0 and ci == 0:
                    nc.sync.dma_start(w2_sb[:, 4:8, :], w2h3[:, 4:8, :])
                    nc.sync.dma_start(lt_sb[:], lt[:])
                    nc.sync.dma_start(cpk_sb[:], cpk[:])
                    nc.sync.dma_start(
                        w1_sb[:, 0:4, 0:512], w1h3[:, 0:4, 0:512]
                    )
                    nc.sync.dma_start(
                        w2_sb[:, 0:4, 0:512], w2h3[:, 0:4, 0:512]
                    )
                    nc.sync.dma_start(gh_sb[:, :, 0:512], gh3[:, :, 0:512])
                    nc.sync.dma_start(gl_sb[:, :, 0:512], gl3[:, :, 0:512])
                xlu_sb = xin.tile([128, KT1, NCHUNK], F8, tag="xlu")
                nc.sync.dma_start(xlu_sb[:], xlu3[:, :, nsl])
                xlm_sb = xin.tile([128, KT1, NCHUNK], F8, tag="xlm")
                nc.sync.dma_start(xlm_sb[:], xlm3[:, :, nsl])
                if rep == 0 and ci == 0:
                    nc.sync.dma_start(gh_sb[:, :, 512:D], gh3[:, :, 512:D])
                    nc.sync.dma_start(gl_sb[:, :, 512:D], gl3[:, :, 512:D])

                # --- z-path: zp = (SW*W) @ x in DoubleRow fp8; square on
                # ScalarE straight out of PSUM into packed fp8 tiles ---
                zq_sb = zqp.tile([128, MT1, NCHUNK], F8)

                def z_tile(mg):
                    half, m = divmod(mg, D // 128)
                    xh_sb, w_sb = ((xhu_sb, w1_sb), (xhm_sb, w2_sb))[half]
                    zp = zps.tile([128, NCHUNK], F32, tag="zp")
                    p0 = m // 2   # k-tile pairs below the diagonal are zero
                    for t in range(p0, KT1 // 2):
                        nc.tensor.matmul(
                            zp[:],
                            lhsT=w_sb[:, 2 * t : 2 * t + 2, ts(m, 128)],
                            rhs=xh_sb[:, 2 * t : 2 * t + 2, :],
                            start=(t == p0),
                            stop=(t == KT1 // 2 - 1),
                            perf_mode=DR,
                        )
                    # zq = ((zp + SW*c) * SQ)^2, quantized to fp8
                    nc.scalar.activation(
                        zq_sb[:, mg, :],
                        zp[:],
                        AF.Square,
                        bias=csq_sb[:, mg : mg + 1],
                        scale=SQ,
                    )

                # --- S1 = r^T x (DoubleRow matvecs over x) ---
                s1p = s1ps.tile([1, NCHUNK], F32, tag="s1")

                def s1_part(t):
                    xh_sb = xhu_sb if t < KT1 // 2 else xhm_sb
                    tt = t if t < KT1 // 2 else t - KT1 // 2
                    nc.tensor.matmul(
                        s1p[:],
                        lhsT=rr_v[:, 2 * t : 2 * t + 2, 0:1],
                        rhs=xh_sb[:, 2 * tt : 2 * tt + 2, :],
                        start=(t == 0),
                        stop=(t == KT3 // 2 - 1),
                        perf_mode=DR,
                    )

                # 64*mu into slot 0 of the DoubleRow rank-1 rhs; slot 1 = 1
                mu2 = scal.tile([1, 2, NCHUNK], F8, tag="mu2")

                def mu_ops():
                    nc.vector.memset(mu2[:, 1, :], 1.0)
                    nc.vector.tensor_scalar(
                        out=mu2[:, 0, :],
                        in0=s1p[:],
                        scalar1=SX / (2 * D * SW),
                        scalar2=sc_sb,
                        op0=ALU.mult,
                        op1=ALU.add,
                    )

                # --- y-path + staging copies to SBUF ---
                ysb = ysbp.tile([128, MT3, NCHUNK], F32)
                yp_last = {}

                def y_mtile(m):
                    yp = yps.tile([128, NCHUNK], F32, tag="yp")
                    first = True
                    for ga_sb, xa in (
                        (gh_sb, (xhu_sb, xhm_sb)),
                        (gl_sb, (xhu_sb, xhm_sb)),
                        (gh_sb, (xlu_sb, xlm_sb)),
                    ):
                        for t in range(KT3 // 2):
                            x_sb = xa[0] if t < KT1 // 2 else xa[1]
                            tt = t if t < KT1 // 2 else t - KT1 // 2
                            nc.tensor.matmul(
                                yp[:],
                                lhsT=ga_sb[:, 2 * t : 2 * t + 2, ts(m, 128)],
                                rhs=x_sb[:, 2 * tt : 2 * tt + 2, :],
                                start=first,
                                stop=False,
                                perf_mode=DR,
                            )
                            first = False
                    # rank-1 update: yp += (SX*mu)*(-wbar_m) + 1*(SX*d2_m)
                    nc.tensor.matmul(
                        yp[:],
                        lhsT=lt_v[:, 2 * m : 2 * m + 2, :],
                        rhs=mu2[:],
                        start=False,
                        stop=True,
                        perf_mode=DR,
                        skip_group_check=True,
                    )
                    if m < 6:
                        nc.vector.tensor_copy(out=ysb[:, m, :], in_=yp[:])
                    else:
                        yp_last[m] = yp

                if rep == 0 and ci == 0:
                    # weights still streaming in: follow the DMA arrival order
                    for mg in (7, 6, 5, 4):
                        z_tile(mg)
                    for t in range(4):
                        s1_part(t)
                    for mg in (15, 14, 13, 12):
                        z_tile(mg)
                    for t in range(4, KT3 // 2):
                        s1_part(t)
                    for mg in (3, 2, 1, 0, 11, 10, 9, 8):
                        z_tile(mg)
                    mu_ops()
                    for m in range(4):
                        y_mtile(m)
                else:
                    # interleave z tiles between y tiles so ScalarE squares
                    # never gate the zps PSUM banks
                    for mg in range(4):
                        z_tile(mg)
                    for t in range(4):
                        s1_part(t)
                    for mg in range(4, 8):
                        z_tile(mg)
                    for t in range(4, 8):
                        s1_part(t)
                    mu_ops()
                    y_mtile(0)
                    for mg in range(8, 10):
                        z_tile(mg)
                    y_mtile(1)
                    for mg in range(10, 12):
                        z_tile(mg)
                    y_mtile(2)
                    for mg in range(12, 14):
                        z_tile(mg)
                    y_mtile(3)
                    for mg in range(14, 16):
                        z_tile(mg)

                # --- S2 = 1^T zq (DoubleRow matvecs over squared z) ---
                s2p = s2ps.tile([1, NCHUNK], F32, tag="s2")
                for t in range(MT1 // 2):
                    nc.tensor.matmul(
                        s2p[:],
                        lhsT=ones2[:, :, 0:1],
                        rhs=zq_sb[:, 2 * t : 2 * t + 2, :],
                        start=(t == 0),
                        stop=(t == MT1 // 2 - 1),
                        perf_mode=DR,
                    )

                # --- mu^2, var, rs/SX while the rest of the y-path runs ---
                musq = scal.tile([1, NCHUNK], F32, tag="musq")
                nc.scalar.activation(
                    musq[:], mu2[:, 0, :], AF.Square, scale=1.0 / SX
                )
                var = scal.tile([1, NCHUNK], F32, tag="var")
                nc.vector.scalar_tensor_tensor(
                    out=var[:],
                    in0=s2p[:],
                    scalar=1.0 / (2 * D * (SW * SQ) ** 2),
                    in1=musq[:],
                    op0=ALU.mult,
                    op1=ALU.subtract,
                )
                sd = scal.tile([1, NCHUNK], F32, tag="sd")
                nc.scalar.activation(
                    sd[:], var[:], AF.Sqrt, bias=eps_sb[:], scale=SX * SX
                )
                rstf = scal.tile([1, NCHUNK], F32, tag="rstf")
                nc.vector.reciprocal(rstf[:], sd[:])
                rst = scal.tile([1, NCHUNK], BF16, tag="rst")
                nc.vector.tensor_copy(out=rst[:], in_=rstf[:])

                y_mtile(4)
                y_mtile(5)

                # --- broadcast rs/SX across partitions (outer product) ---
                bp = bps.tile([128, NCHUNK], F32, tag="bp")
                nc.tensor.matmul(
                    bp[:],
                    lhsT=onesf[:],
                    rhs=rst[:],
                    start=True,
                    stop=True,
                )

                bp_sb = outp.tile([128, NCHUNK], F32, tag="bp_sb")
                nc.vector.tensor_copy(out=bp_sb[:], in_=bp[:])

                # --- epilogue: scale by rs/SX, bias + exact-erf GELU; the
                # last two y m-tiles overlap the first six epilogues ---
                def epi(m):
                    t1 = outp.tile([128, NCHUNK], F32, tag="t1")
                    src_ap = ysb[:, m, :] if m < 6 else yp_last[m][:]
                    nc.vector.tensor_mul(t1[:], src_ap, bp_sb[:])
                    o_sb = opool.tile([128, NCHUNK], BF16, tag="o")
                    nc.scalar.activation(
                        o_sb[:], t1[:], AF.Gelu, bias=cb2_sb[:, m : m + 1]
                    )
                    nc.sync.dma_start(out[ts(m, 128), nsl], o_sb[:])

                epi(0)
                epi(1)
                epi(2)
                y_mtile(6)
                epi(3)
                epi(4)
                epi(5)
                epi(6)
                y_mtile(7)
                if ci == NCH - 1 and rep == repeat - 1:
                    for hs in (slice(0, 256), slice(256, 512)):
                        t1 = outp.tile([128, NCHUNK], F32, tag="t1")
                        nc.vector.tensor_mul(
                            t1[:, hs], yp_last[7][:, hs], bp_sb[:, hs]
                        )
                        o_sb = opool.tile([128, NCHUNK], BF16, tag="o")
                        nc.scalar.activation(
                            o_sb[:, hs], t1[:, hs], AF.Gelu,
                            bias=cb2_sb[:, 7:8],
                        )
                        nc.sync.dma_start(
                            out[ts(7, 128), n0 + hs.start : n0 + hs.stop],
                            o_sb[:, hs],
                        )
                else:
                    epi(7)
    split_multi_waits(nc)
    return nc


def q8(a):
    return np.clip(np.asarray(a, np.float32), -240.0, 240.0).astype(nf8)


def fold_weights(inputs):
    f32 = np.float32
    d = D
    w_qkv1 = np.asarray(inputs["w_qkv1"], f32)
    w_qkv2 = np.asarray(inputs["w_qkv2"], f32)
    b_qkv1 = np.asarray(inputs["b_qkv1"], f32)
    b_qkv2 = np.asarray(inputs["b_qkv2"], f32)
    w_o1 = np.asarray(inputs["w_o1"], f32)
    w_o2 = np.asarray(inputs["w_o2"], f32)
    b_o1 = np.asarray(inputs["b_o1"], f32)
    b_o2 = np.asarray(inputs["b_o2"], f32)
    w_proj = np.asarray(inputs["w_proj"], f32)
    b_proj = np.asarray(inputs["b_proj"], f32)
    g = np.asarray(inputs["ln_g"], f32)
    lb = np.asarray(inputs["ln_b"], f32)

    wv1, bv1 = w_qkv1[2 * d :], b_qkv1[2 * d :]
    wv2, bv2 = w_qkv2[2 * d :], b_qkv2[2 * d :]
    W1 = w_o1 @ wv1
    c1 = w_o1 @ bv1 + b_o1
    W2 = w_o2 @ wv2
    c2 = w_o2 @ bv2 + b_o2
    Wg = w_proj * g[None, :]
    G = np.concatenate([Wg[:, :d] @ W1, Wg[:, d:] @ W2], axis=1)  # [d, 2d]
    cvec = np.concatenate([c1, c2])
    d2 = Wg @ cvec
    wbar = Wg.sum(axis=1)
    b2 = w_proj @ lb + b_proj

    Gs = G.T * SX                      # [2d, d] lhsT layout
    gh = q8(Gs)
    gl = q8(Gs - gh.astype(f32))

    def chol_factor(W):
        # sum_f z_f^2 = x^T (W^T W) x = ||L^T x||^2; lower-tri L halves the
        # k-tiles of the stats matmul. Jitter guards near-singular C.
        C = W.T.astype(np.float64) @ W.astype(np.float64)
        scale = np.trace(C) / d
        for jit in (0.0, 1e-12, 1e-9, 1e-6):
            try:
                return np.linalg.cholesky(C + jit * scale * np.eye(d))
            except np.linalg.LinAlgError:
                continue
        raise np.linalg.LinAlgError("cholesky failed")

    w1h = q8(chol_factor(W1).astype(f32) * SW)
    w2h = q8(chol_factor(W2).astype(f32) * SW)
    rvec = np.concatenate([W1.sum(axis=0), W2.sum(axis=0)]) * SW
    rrv = np.zeros((128, 16, 16), f32)
    rrv[:, :, 0] = rvec.reshape(16, 128).T
    ltv = np.empty((1, 2 * MT3, 128), f32)
    ltv[0, 0::2, :] = (-wbar).reshape(MT3, 128)
    ltv[0, 1::2, :] = (SX * d2).reshape(MT3, 128)
    cpkv = np.zeros((128, MT3 + MT1 + 1), f32)
    cpkv[:, 0:MT3] = b2.reshape(MT3, 128).T
    cpkv[:, MT3 : MT3 + MT1] = (2.0 * cvec).reshape(MT1, 128).T
    cpkv[0, MT3 + MT1] = SX * cvec.sum() / (2 * d)
    return {
        "gh": gh,
        "gl": gl,
        "w1h": w1h,
        "w2h": w2h,
        "rr": q8(rrv.reshape(128, 256)),
        "lt": q8(ltv.reshape(1, 2 * MT3 * 128)),
        "cpk": cpkv,
        "_czero": not np.any(cvec),
    }


_CACHED_NC = None


def _get_program(czero=True):
    global _CACHED_NC
    if _CACHED_NC is None:
        _CACHED_NC = build_program()
    return _CACHED_NC


def run(inputs, trace=False):
    """Build per-core shards, run on 8 cores, return (full_out, results)."""
    x_u = np.asarray(inputs["x_u"], np.float32)
    x_m = np.asarray(inputs["x_m"], np.float32)
    shared = fold_weights(inputs)
    czero = shared.pop("_czero")
    xuT = np.ascontiguousarray(x_u.T)  # [D, B] f32
    xmT = np.ascontiguousarray(x_m.T)
    xhuT = q8(xuT)
    xhmT = q8(xmT)
    xluT = q8(xuT - xhuT.astype(np.float32))
    xlmT = q8(xmT - xhmT.astype(np.float32))

    in_maps = []
    for c in range(N_CORES):
        sl = slice(c * BC, (c + 1) * BC)
        m = dict(shared)
        m["xhu"] = np.ascontiguousarray(xhuT[:, sl])
        m["xhm"] = np.ascontiguousarray(xhmT[:, sl])
        m["xlu"] = np.ascontiguousarray(xluT[:, sl])
        m["xlm"] = np.ascontiguousarray(xlmT[:, sl])
        in_maps.append(m)

    nc = _get_program(czero=czero)
    res = run_bass_kernel_spmd(
        nc, in_maps, list(range(N_CORES)), trace=trace
    )
    out = np.empty((B, D), np.float32)
    for c in range(N_CORES):
        out[c * BC : (c + 1) * BC, :] = res.results[c]["outT"].T.astype(np.float32)
    return out, res


def kernel(**inputs) -> np.ndarray:
    out, _ = run(inputs, trace=False)
    return out
